# revision 20
# baseline (speedup 1.0000x reference)
"""Sparse-attention Trainium2 kernel (8 NeuronCores, sequence-parallel).

Problem (hardcoded): B=1, S=4096, H=1024, NH=16, D=64, K=32.

Sharding: fully sequence-parallel. Core c owns query rows [512c, 512c+512).
It computes q/k/v for its own rows against the FULL weight matrices,
publishes its k|v rows via an 8-way AllGather, then dma_gathers per-query
k/v rows for ALL 16 heads at once (4 KB/descriptor), computes the sparse
attention for its rows and the o-projection. Host concatenates row slices.

Phase-B dataflow (v2, rebalanced vs the 494us baseline):
- q is kept in SBUF from phase A and replicated 16->128 partitions per tile
  by a PE matmul against a static selection matrix (selrep), killing the
  per-tile SWDGE qrep gather (Pool + DMA traffic).
- softmax normalization moved BEFORE the AV product: r16 = 1/den on 16
  partitions is partition-replicated to 128 via a PE matmul (s16t), then
  e2n = e2 * r2rep on DVE (free 128) replaces the [16,1024] A-normalize.
- AV "flip": psAT[ch,q] = sum_p W[p,ch] * s16[p,q] -- moving=W chunk,
  stationary=s16 (constant!), out free=16. 32 small matmuls pipeline at
  ~30ns spacing, replacing the old 8x630ns psA + 8 PE transposes; the
  PSUM result IS A^T so it copies straight into the o-proj group buffer.
- two-stage software pipeline: per iteration emit gather(i+1)/qrep(i+1),
  logits+exp+den+recip for tile i, and e2n/W/psAT for tile i-1, so the
  DVE never head-of-line blocks on the exp->den->recip round trip.

Measured: 454us (vs 489us baseline), rel err 5.6e-3. Phase A 0-75us
(PE-bound projections), AllGather 75-155us (ALL engines idle), phase B
160-450us at ~9.0us/tile with DVE 100% busy (~8.1us/tile: t1 mul 2.27 +
tree 2.82 + W mul 2.29 + e2n/recip/small). DVE does ~2.2 passes over the
64MB/core of gathered kv at ~490GB/s -- that is the structural floor of
this dataflow. PE after the flip ~4.2; DMA ~6.0; Pool = descgen only.

Lessons measured on HW (do NOT retry without new information):
- remote_dma_broadcast peer-to-peer kv exchange (to kill the ~80us
  collective window): NEFF dies with INTERNAL error even with all sem
  waits removed -- the remote-DMA/SWDGE-broadcast transport appears
  nonfunctional under this axon-tunneled runtime. The full epoch-safe
  handshake design is in git-history/transcript if transport ever works.
- num_swdge_queues=2 with gathers alternating queues: INTERNAL error.
- Splitting xT/w3 into per-kc tiles for earlier matmul start: 462us
  (slightly WORSE than monolithic, possibly variance; reverted).
- 4-way chunked AllGather: worse (~26us fixed per chunk, prior session).
- Pool-engine tensor ops to offload DVE: impossible -- InstTensorTensor
  is not in the mlp/attnmlp GPSIMD libraries that dma_gather needs, so
  alternating them would thrash mid-kernel library reloads every tile.
- fp8 kv: DVE 2x mode requires 2-byte dtypes, so fp8 operands halve DVE
  throughput -- costs more than the DMA it saves (prior session: ACT
  upcast also too slow). kvsel gathers need single_packet=False.
- PE logits (k^T transpose-gather) always land as [pos, q'] outer
  products needing a per-partition diagonal read no AP can express.
"""

import os
from contextlib import ExitStack

import numpy as np
import ml_dtypes

S, H, NH, D, K = 4096, 1024, 16, 64, 32
NCORES = 8
SC = S // NCORES            # 512 rows per core
QT = 16                     # queries per attention tile
NTB = SC // QT              # 32 attention tiles per core
NST = SC // 128             # 4 projection s-tiles per core
CH = NH * D                 # 1024 kv channels per tensor
ROW = 2 * CH                # 2048 bf16 elems per kv row (4 KB)
NCC = K // 8                # 4 slot chunks per tile
BF16 = ml_dtypes.bfloat16

_nc_cache = None


def build_nc(mode="full"):
    import concourse.bass as bass
    import concourse.mybir as mybir
    import concourse.tile as tile
    from concourse import bacc
    from concourse.tile_rust import add_dep_helper
    from concourse.bass import ts, ds

    dt = mybir.dt
    nc = bacc.Bacc("TRN2", target_bir_lowering=False, debug=False,
                   num_devices=NCORES)

    xT = nc.dram_tensor("xT", [H, SC], dt.bfloat16, kind="ExternalInput")
    w3T = nc.dram_tensor("w3T", [H, 3 * CH], dt.bfloat16, kind="ExternalInput")
    woT = nc.dram_tensor("woT", [CH, H], dt.bfloat16, kind="ExternalInput")
    gb = nc.dram_tensor("gb", [128, NTB * 4 * NH], dt.float32, kind="ExternalInput")
    idx16 = nc.dram_tensor("idx16", [128, NTB * (QT * K // 16)], dt.int16,
                           kind="ExternalInput")
    s16d = nc.dram_tensor("s16", [128, 16], dt.bfloat16, kind="ExternalInput")
    s16td = nc.dram_tensor("s16t", [16, 128], dt.float32, kind="ExternalInput")
    selrepd = nc.dram_tensor("selrep", [128, 8 * 128], dt.bfloat16,
                             kind="ExternalInput")
    outd = nc.dram_tensor("out", [SC, H], dt.float32, kind="ExternalOutput")
    kv_loc = nc.dram_tensor("kv_loc", [SC, ROW], dt.bfloat16, kind="Internal")
    kv_full = nc.dram_tensor("kv_full", [S, ROW], dt.bfloat16, kind="Internal",
                             addr_space="Shared")

    EXP = mybir.ActivationFunctionType.Exp

    with ExitStack() as ctx:
        tc = ctx.enter_context(tile.TileContext(nc))
        const = ctx.enter_context(tc.tile_pool(name="const", bufs=1))

        kv_pool = ctx.enter_context(tc.tile_pool(name="kvout", bufs=2))
        ps_big = ctx.enter_context(tc.tile_pool(name="ps_big", bufs=2, space="PSUM"))
        ps_q = ctx.enter_context(tc.tile_pool(name="ps_q", bufs=2, space="PSUM"))
        ps_sm = ctx.enter_context(tc.tile_pool(name="ps_sm", bufs=2, space="PSUM"))

        # ---- phase-A weights first: the kv matmuls gate the collective ----
        wa = tc.tile_pool(name="wa", bufs=1)
        wap = wa.__enter__()
        xT_sb = wap.tile([128, 8, SC], dt.bfloat16)           # 1 MB, phase A only
        for kc in range(8):
            nc.sync.dma_start(xT_sb[:, kc, :], xT[ts(kc, 128), :])
        w3_sb = wap.tile([128, 8, 3 * CH], dt.bfloat16)       # 6 MB, phase A only
        for kc in range(8):
            nc.sync.dma_start(w3_sb[:, kc, ds(CH, 2 * CH)],
                              w3T[ts(kc, 128), ds(CH, 2 * CH)])
        for kc in range(8):
            nc.sync.dma_start(w3_sb[:, kc, ds(0, CH)], w3T[ts(kc, 128), ds(0, CH)])

        # ---- small resident tensors ----
        idx_sb = const.tile([128, NTB * 32], dt.int16)        # 0.25 MB
        nc.sync.dma_start(idx_sb[:], idx16[:, :])
        gb_sb = const.tile([128, NTB, 4 * NH], dt.float32)    # 1 MB
        nc.sync.dma_start(gb_sb[:], gb[:, :])
        s16_sb = const.tile([128, 16], dt.bfloat16)
        nc.sync.dma_start(s16_sb[:], s16d[:, :])
        s16t_sb = const.tile([16, 128], dt.float32)
        nc.sync.dma_start(s16t_sb[:], s16td[:, :])
        selrep_sb = const.tile([128, 8, 128], dt.bfloat16)
        nc.sync.dma_start(selrep_sb[:],
                          selrepd[:, :].rearrange("p (g m) -> p g m", g=8))
        wo_sb = const.tile([128, 8, H], dt.bfloat16)          # 2 MB
        for chn in range(8):
            nc.sync.dma_start(wo_sb[:, chn, :], woT[ts(chn, 128), :])
        q_sb = const.tile([128, NST, CH], dt.bfloat16)        # 1 MB, lives all of B

        # ---- phase A: k/v first (AllGather can start early), then q ----
        kv_stores = []
        for st in range(NST):
            kvt_cur = None
            for pj in (1, 2):         # k then v
                ps = ps_big.tile([128, CH], dt.float32, tag="psb")
                for n in range(2):
                    for kc in range(8):
                        nc.tensor.matmul(
                            ps[:, ts(n, 512)],
                            xT_sb[:, kc, ts(st, 128)],
                            w3_sb[:, kc, ds(pj * CH + n * 512, 512)],
                            start=(kc == 0), stop=(kc == 7))
                if pj == 1:
                    kvt_cur = kv_pool.tile([128, 2, CH], dt.bfloat16, tag="kvt")
                    nc.scalar.copy(kvt_cur[:, 0, :], ps[:])
                else:
                    nc.scalar.copy(kvt_cur[:, 1, :], ps[:])
                    kv_stores.append(nc.sync.dma_start(
                        kv_loc[ts(st, 128), :],
                        kvt_cur[:].rearrange("p a b -> p (a b)")))
        cc_i = nc.gpsimd.collective_compute(
            "AllGather", mybir.AluOpType.bypass,
            replica_groups=[list(range(NCORES))],
            ins=[kv_loc[:, :]], outs=[kv_full[:, :]])
        for stn in kv_stores:
            add_dep_helper(cc_i.ins, stn.ins, sync=True, reason="cc after kv stores")
        cc_insts = [cc_i]
        for st in range(NST):         # q after all k/v (overlaps AllGather)
            ps = ps_big.tile([128, CH], dt.float32, tag="psb")
            for n in range(2):
                for kc in range(8):
                    nc.tensor.matmul(
                        ps[:, ts(n, 512)],
                        xT_sb[:, kc, ts(st, 128)],
                        w3_sb[:, kc, ds(n * 512, 512)],
                        start=(kc == 0), stop=(kc == 7))
            nc.scalar.copy(q_sb[:, st, :], ps[:])
        wa.__exit__(None, None, None)
        gat = ctx.enter_context(tc.tile_pool(name="gat", bufs=6))
        big = ctx.enter_context(tc.tile_pool(name="big", bufs=2))
        small = ctx.enter_context(tc.tile_pool(name="small", bufs=4))
        atg_pool = ctx.enter_context(tc.tile_pool(name="atg", bufs=2))
        outp = ctx.enter_context(tc.tile_pool(name="outp", bufs=1))

        # ---- phase B: per-tile sparse attention, 2-stage software pipe ----
        NT = NTB if mode != "proj" else 0
        kvsel = {}      # tile -> gathered k|v rows [128, NCC, ROW]
        qrep = {}       # tile -> q replicated to 128 partitions
        e2 = {}         # tile -> exp(logits), pair-expanded
        r2rep = {}      # tile -> 1/den replicated to 128 partitions
        atg_cur = None

        def emit_gather(t):
            kvsel[t] = gat.tile([128, NCC, ROW], dt.bfloat16, tag="kvsel", name="kvsel")
            g = nc.gpsimd.dma_gather(
                out_ap=kvsel[t][:], in_ap=kv_full[:, :],
                idxs_ap=idx_sb[:, ds(t * 32, 32)],
                num_idxs=QT * K, num_idxs_reg=QT * K,
                elem_size=ROW, single_packet=False)
            for cci in cc_insts:
                add_dep_helper(g.ins, cci.ins, sync=True, reason="gather after cc")

        def emit_qrep(t):
            st, g16 = t // 8, t % 8
            qrep[t] = small.tile([128, CH], dt.bfloat16, tag="qrep", name="qrep")
            for n in range(2):
                psq = ps_q.tile([128, 512], dt.float32, tag="psq")
                nc.tensor.matmul(psq[:], selrep_sb[:, g16, :],
                                 q_sb[:, st, ts(n, 512)], start=True, stop=True)
                nc.scalar.copy(qrep[t][:, ts(n, 512)], psq[:])

        def emit_stage_a(t):
            # logits: t1 = q*k, halving-tree d-reduction (pure 2x TT ops)
            t1 = big.tile([128, NCC, CH], dt.bfloat16, tag="t1")
            k_ap = kvsel[t][:, :, 0:CH]
            k_ap2, q_ap2 = bass.broadcast_tensor_aps(
                k_ap, qrep[t][:].rearrange("p (o c) -> p o c", o=1))
            nc.vector.tensor_mul(t1[:], k_ap2, q_ap2)
            t1v = t1[:].rearrange("p c (h d) -> p (c h) d", d=D)
            nc.vector.tensor_add(t1v[:, :, 0:32], t1v[:, :, 0:32], t1v[:, :, 32:64])
            nc.vector.tensor_add(t1v[:, :, 0:16], t1v[:, :, 0:16], t1v[:, :, 16:32])
            nc.vector.tensor_add(t1v[:, :, 0:8], t1v[:, :, 0:8], t1v[:, :, 8:16])
            nc.vector.tensor_add(t1v[:, :, 0:4], t1v[:, :, 0:4], t1v[:, :, 4:8])
            nc.vector.tensor_add(t1v[:, :, 0:2], t1v[:, :, 0:2], t1v[:, :, 2:4])
            lgt = small.tile([128, 4 * NH], dt.float32, tag="lgt")
            lgtv = lgt[:].rearrange("p (g o) -> p g o", o=1)
            nc.vector.tensor_add(lgtv, t1v[:, :, 0:1], t1v[:, :, 1:2])
            nc.vector.tensor_add(lgt[:], lgt[:], gb_sb[:, t, :])

            # e2 = exp(logits), pair-expanded (ACT)
            e2[t] = small.tile([128, NCC, NH, 2], dt.bfloat16, tag="e2", name="e2")
            lgt4 = lgt[:].rearrange("p (c h o) -> p c h o", c=NCC, o=1)
            e2a, lgt4b = bass.broadcast_tensor_aps(e2[t][:], lgt4)
            nc.scalar.activation(e2a, lgt4b, EXP)

            # denominator on PE (both pair lanes kept)
            psd = ps_sm.tile([16, 2 * NH], dt.float32, tag="pss")
            for cc in range(NCC):
                nc.tensor.matmul(psd[:], s16_sb[:],
                                 e2[t][:, cc, :, :].rearrange("p h w -> p (h w)"),
                                 start=(cc == 0), stop=(cc == NCC - 1))
            return psd

        def emit_recip_rep(t, psd):
            # r16 = 1/den on 16 partitions, then replicate to 128 via PE
            r16 = small.tile([16, 2 * NH], dt.float32, tag="r16")
            nc.vector.reciprocal_approx_fast(r16[:], psd[:])
            psr = ps_sm.tile([128, 2 * NH], dt.float32, tag="pss")
            nc.tensor.matmul(psr[:], s16t_sb[:], r16[:], start=True, stop=True)
            r2rep[t] = small.tile([128, 1, NH, 2], dt.bfloat16, tag="r2rep", name="r2rep")
            nc.scalar.copy(r2rep[t][:].rearrange("p o h w -> p (o h w)"), psr[:])

        def emit_stage_b(t):
            nonlocal atg_cur
            st, g16 = t // 8, t % 8
            # e2n = e2 * (1/den) -- normalized weights, pair-expanded
            e2n = small.tile([128, NCC, NH, 2], dt.bfloat16, tag="e2n")
            e_in, r_in = bass.broadcast_tensor_aps(e2[t][:], r2rep[t][:])
            nc.vector.tensor_mul(e2n[:], e_in, r_in)
            # W = v * e2n (bcast over d, pair-expanded 2x)
            W = big.tile([128, NCC, CH], dt.bfloat16, tag="W")
            v_ap2, e_ap2 = bass.broadcast_tensor_aps(
                kvsel[t][:, :, CH:ROW].rearrange(
                    "p c (h dd w) -> p c h dd w", dd=32, w=2),
                e2n[:].rearrange("p c h (dd w) -> p c h dd w", dd=1, w=2))
            nc.vector.tensor_mul(
                W[:].rearrange("p c (h dd w) -> p c h dd w", dd=32, w=2),
                v_ap2, e_ap2)
            # AV flip: psAT[ch, q] = sum_p W[p, ch] s16[p, q]; constant
            # stationary, out free=16 -> 32 matmuls pipelining at ~30ns
            psat = ps_sm.tile([128, 8, QT], dt.float32, tag="pss")
            for chk in range(8):
                for cc in range(NCC):
                    nc.tensor.matmul(psat[:, chk, :],
                                     W[:, cc, ts(chk, 128)], s16_sb[:],
                                     start=(cc == 0), stop=(cc == NCC - 1))
            if g16 == 0:
                atg_cur = atg_pool.tile([128, 8, 128], dt.bfloat16, tag="atg")
            nc.scalar.copy(atg_cur[:, :, ds(QT * g16, QT)], psat[:])
            # o-proj per group of 8 tiles (128 query rows)
            if g16 == 7:
                psP = ps_big.tile([128, H], dt.float32, tag="psb")
                for n in range(2):
                    for chk in range(8):
                        nc.tensor.matmul(psP[:, ts(n, 512)], atg_cur[:, chk, :],
                                         wo_sb[:, chk, ts(n, 512)],
                                         start=(chk == 0), stop=(chk == 7))
                ot = outp.tile([128, H], dt.float32, tag="ot")
                nc.scalar.copy(ot[:], psP[:])
                nc.sync.dma_start(outd[ts(st, 128), :], ot[:])

        # gathers prefetched 2 iterations ahead: gather latency (descgen +
        # DMA queue + transfer + sem) is ~a full tile period, so +1 ahead
        # left the t1 mul ~0.8us short every tile
        for t0 in range(min(2, NT)):
            emit_gather(t0)
            emit_qrep(t0)
        for i in range(NT):
            if i + 2 < NT:
                emit_gather(i + 2)
                emit_qrep(i + 2)
            psd = emit_stage_a(i)
            if i > 0:
                emit_stage_b(i - 1)
            emit_recip_rep(i, psd)
        if NT > 0:
            emit_stage_b(NT - 1)

    nc.compile()
    return nc


def prep_inputs(x, idx, valid, geo_bias, Wq, Wk, Wv, Wo, bo, epoch=1):
    """Host-side shard prep. Returns (in_maps, bo_f32)."""
    x = np.asarray(x)
    idx = np.asarray(idx)
    geo_bias = np.asarray(geo_bias)
    Wq, Wk, Wv, Wo = (np.asarray(w) for w in (Wq, Wk, Wv, Wo))
    bo = np.asarray(bo, dtype=np.float32)

    x2 = x.reshape(S, H)
    scale = np.float32(1.0 / np.sqrt(D))
    w3T = np.ascontiguousarray(
        np.concatenate([(Wq * scale).T, Wk.T, Wv.T], axis=1).astype(BF16))
    woT = np.ascontiguousarray(Wo.T.astype(BF16))
    s16 = np.zeros((128, 16), dtype=BF16)
    s16[np.arange(128), np.arange(128) % 16] = 1
    s16t = np.zeros((16, 128), dtype=np.float32)
    s16t[np.arange(128) % 16, np.arange(128)] = 1
    # selrep[r, g, m] = 1 iff r == 16 g + m%16  (q replication matrices)
    selrep = np.zeros((128, 8, 128), dtype=BF16)
    for g in range(8):
        m = np.arange(128)
        selrep[16 * g + m % 16, g, m] = 1
    selrep = np.ascontiguousarray(selrep.reshape(128, 8 * 128))

    in_maps = []
    for c in range(NCORES):
        rb = c * SC
        xTc = np.ascontiguousarray(x2[rb:rb + SC].T.astype(BF16))

        # gather indices: tile t, pos = j*16 + q -> idx[rb + t*16 + q, j]
        idxc = np.empty((16, NTB * 32), dtype=np.int16)
        for t in range(NTB):
            blk = idx[rb + t * QT: rb + (t + 1) * QT, :]      # [16 q, 32 j]
            lin = np.asarray(blk.T.reshape(-1))                # pos = j*16+q
            idxc[:, t * 32:(t + 1) * 32] = lin.reshape(32, 16).T.astype(np.int16)
        idxc = np.ascontiguousarray(np.tile(idxc, (8, 1)))

        # geo bias: gb[p=(b,qq), t, cc*16+h] = geo_bias[h, rb+t*16+qq, cc*8+b]
        gg = geo_bias[:, rb:rb + SC, :]                        # [h, 512, j]
        g2 = gg.reshape(NH, NTB, QT, NCC, 8)                   # [h, t, qq, cc, b]
        gbt = g2.transpose(4, 2, 1, 3, 0).reshape(128, NTB * 4 * NH)
        gbt = np.ascontiguousarray(gbt, dtype=np.float32)

        in_maps.append({
            "xT": xTc,
            "w3T": w3T,
            "woT": woT,
            "gb": gbt,
            "idx16": idxc,
            "s16": s16,
            "s16t": s16t,
            "selrep": selrep,
        })
    return in_maps, bo


_epoch = 0


def kernel(x, idx, valid, geo_bias, Wq, Wk, Wv, Wo, bo):
    global _nc_cache, _epoch
    from concourse.bass_utils import run_bass_kernel_spmd

    if _nc_cache is None:
        _nc_cache = build_nc()
    nc = _nc_cache

    _epoch += 1
    in_maps, bo_f32 = prep_inputs(x, idx, valid, geo_bias, Wq, Wk, Wv, Wo, bo,
                                  epoch=_epoch)
    res = run_bass_kernel_spmd(nc, in_maps, core_ids=list(range(NCORES)),
                               trace=bool(int(os.environ.get("KTRACE", "0"))))
    out = np.concatenate([r["out"] for r in res.results], axis=0)
    out = out + bo_f32[None, :]
    if res.exec_time_ns is not None:
        kernel.last_exec_time_ns = res.exec_time_ns
    kernel.last_results = res
    return out.reshape(1, S, H).astype(np.float32)


# revision 21
# speedup vs baseline: 1.1115x; 1.1115x over previous
"""Sparse-attention Trainium2 kernel (8 NeuronCores, sequence-parallel).

Problem (hardcoded): B=1, S=4096, H=1024, NH=16, D=64, K=32.

Sharding: fully sequence-parallel. Core c owns query rows [512c, 512c+512).
It computes q/k/v for its own rows against the FULL weight matrices,
publishes its k|v rows via an 8-way AllGather, then dma_gathers per-query
k/v rows for ALL 16 heads at once (4 KB/descriptor), computes the sparse
attention for its rows and the o-projection. Host concatenates row slices.

Phase-B dataflow (v2, rebalanced vs the 494us baseline):
- q is kept in SBUF from phase A and replicated 16->128 partitions per tile
  by a PE matmul against a static selection matrix (selrep), killing the
  per-tile SWDGE qrep gather (Pool + DMA traffic).
- softmax normalization moved BEFORE the AV product: r16 = 1/den on 16
  partitions is partition-replicated to 128 via a PE matmul (s16t), then
  e2n = e2 * r2rep on DVE (free 128) replaces the [16,1024] A-normalize.
- AV "flip": psAT[ch,q] = sum_p W[p,ch] * s16[p,q] -- moving=W chunk,
  stationary=s16 (constant!), out free=16. 32 small matmuls pipeline at
  ~30ns spacing, replacing the old 8x630ns psA + 8 PE transposes; the
  PSUM result IS A^T so it copies straight into the o-proj group buffer.
- two-stage software pipeline: per iteration emit gather(i+1)/qrep(i+1),
  logits+exp+den+recip for tile i, and e2n/W/psAT for tile i-1, so the
  DVE never head-of-line blocks on the exp->den->recip round trip.

Measured: 454us (vs 489us baseline), rel err 5.6e-3. Phase A 0-75us
(PE-bound projections), AllGather 75-155us (ALL engines idle), phase B
160-450us at ~9.0us/tile with DVE 100% busy (~8.1us/tile: t1 mul 2.27 +
tree 2.82 + W mul 2.29 + e2n/recip/small). DVE does ~2.2 passes over the
64MB/core of gathered kv at ~490GB/s -- that is the structural floor of
this dataflow. PE after the flip ~4.2; DMA ~6.0; Pool = descgen only.

Lessons measured on HW (do NOT retry without new information):
- remote_dma_broadcast peer-to-peer kv exchange (to kill the ~80us
  collective window): NEFF dies with INTERNAL error even with all sem
  waits removed -- the remote-DMA/SWDGE-broadcast transport appears
  nonfunctional under this axon-tunneled runtime. The full epoch-safe
  handshake design is in git-history/transcript if transport ever works.
- num_swdge_queues=2 with gathers alternating queues: INTERNAL error.
- Splitting xT/w3 into per-kc tiles for earlier matmul start: 462us
  (slightly WORSE than monolithic, possibly variance; reverted).
- Prefetching gathers 2 iterations ahead (gat bufs 6): 504us, much
  WORSE -- the deeper in-flight gather queue delays the tile at the
  head of the DMA-engine pool. Keep prefetch depth 1.
- 4-way chunked AllGather: worse (~26us fixed per chunk, prior session).
- Pool-engine tensor ops to offload DVE: impossible -- InstTensorTensor
  is not in the mlp/attnmlp GPSIMD libraries that dma_gather needs, so
  alternating them would thrash mid-kernel library reloads every tile.
- fp8 kv: DVE 2x mode requires 2-byte dtypes, so fp8 operands halve DVE
  throughput -- costs more than the DMA it saves (prior session: ACT
  upcast also too slow). kvsel gathers need single_packet=False.
- PE logits (k^T transpose-gather) always land as [pos, q'] outer
  products needing a per-partition diagonal read no AP can express.
"""

import os
from contextlib import ExitStack

import numpy as np
import ml_dtypes

S, H, NH, D, K = 4096, 1024, 16, 64, 32
NCORES = 8
SC = S // NCORES            # 512 rows per core
QT = 16                     # queries per attention tile
NTB = SC // QT              # 32 attention tiles per core
NST = SC // 128             # 4 projection s-tiles per core
CH = NH * D                 # 1024 kv channels per tensor
ROW = 2 * CH                # 2048 bf16 elems per kv row (4 KB)
NCC = K // 8                # 4 slot chunks per tile
BF16 = ml_dtypes.bfloat16

_nc_cache = None


def build_nc(mode="full"):
    import concourse.bass as bass
    import concourse.mybir as mybir
    import concourse.tile as tile
    from concourse import bacc
    from concourse.tile_rust import add_dep_helper
    from concourse.bass import ts, ds

    dt = mybir.dt
    nc = bacc.Bacc("TRN2", target_bir_lowering=False, debug=False,
                   num_devices=NCORES)

    xT = nc.dram_tensor("xT", [H, SC], dt.bfloat16, kind="ExternalInput")
    w3T = nc.dram_tensor("w3T", [H, 3 * CH], dt.bfloat16, kind="ExternalInput")
    woT = nc.dram_tensor("woT", [CH, H], dt.bfloat16, kind="ExternalInput")
    gb = nc.dram_tensor("gb", [128, NTB * 4 * NH], dt.float32, kind="ExternalInput")
    idx16 = nc.dram_tensor("idx16", [128, NTB * (QT * K // 16)], dt.int16,
                           kind="ExternalInput")
    s16d = nc.dram_tensor("s16", [128, 16], dt.bfloat16, kind="ExternalInput")
    s16td = nc.dram_tensor("s16t", [16, 128], dt.float32, kind="ExternalInput")
    selrepd = nc.dram_tensor("selrep", [128, 8 * 128], dt.bfloat16,
                             kind="ExternalInput")
    outd = nc.dram_tensor("out", [SC, H], dt.float32, kind="ExternalOutput")
    kv_loc = nc.dram_tensor("kv_loc", [SC, ROW], dt.bfloat16, kind="Internal")
    kv_full = nc.dram_tensor("kv_full", [S, ROW], dt.bfloat16, kind="Internal",
                             addr_space="Shared")

    EXP = mybir.ActivationFunctionType.Exp

    with ExitStack() as ctx:
        tc = ctx.enter_context(tile.TileContext(nc))
        const = ctx.enter_context(tc.tile_pool(name="const", bufs=1))

        kv_pool = ctx.enter_context(tc.tile_pool(name="kvout", bufs=2))
        ps_big = ctx.enter_context(tc.tile_pool(name="ps_big", bufs=2, space="PSUM"))
        ps_q = ctx.enter_context(tc.tile_pool(name="ps_q", bufs=2, space="PSUM"))
        ps_sm = ctx.enter_context(tc.tile_pool(name="ps_sm", bufs=2, space="PSUM"))

        # ---- phase-A weights first: the kv matmuls gate the collective ----
        wa = tc.tile_pool(name="wa", bufs=1)
        wap = wa.__enter__()
        xT_sb = wap.tile([128, 8, SC], dt.bfloat16)           # 1 MB, phase A only
        for kc in range(8):
            nc.sync.dma_start(xT_sb[:, kc, :], xT[ts(kc, 128), :])
        w3_sb = wap.tile([128, 8, 3 * CH], dt.bfloat16)       # 6 MB, phase A only
        for kc in range(8):
            nc.sync.dma_start(w3_sb[:, kc, ds(CH, 2 * CH)],
                              w3T[ts(kc, 128), ds(CH, 2 * CH)])
        for kc in range(8):
            nc.sync.dma_start(w3_sb[:, kc, ds(0, CH)], w3T[ts(kc, 128), ds(0, CH)])

        # ---- small resident tensors ----
        idx_sb = const.tile([128, NTB * 32], dt.int16)        # 0.25 MB
        nc.sync.dma_start(idx_sb[:], idx16[:, :])
        gb_sb = const.tile([128, NTB, 4 * NH], dt.float32)    # 1 MB
        nc.sync.dma_start(gb_sb[:], gb[:, :])
        s16_sb = const.tile([128, 16], dt.bfloat16)
        nc.sync.dma_start(s16_sb[:], s16d[:, :])
        s16t_sb = const.tile([16, 128], dt.float32)
        nc.sync.dma_start(s16t_sb[:], s16td[:, :])
        selrep_sb = const.tile([128, 8, 128], dt.bfloat16)
        nc.sync.dma_start(selrep_sb[:],
                          selrepd[:, :].rearrange("p (g m) -> p g m", g=8))
        wo_sb = const.tile([128, 8, H], dt.bfloat16)          # 2 MB
        for chn in range(8):
            nc.sync.dma_start(wo_sb[:, chn, :], woT[ts(chn, 128), :])
        q_sb = const.tile([128, NST, CH], dt.bfloat16)        # 1 MB, lives all of B

        # ---- phase A: k/v first (AllGather can start early), then q ----
        kv_stores = []
        for st in range(NST):
            kvt_cur = None
            for pj in (1, 2):         # k then v
                ps = ps_big.tile([128, CH], dt.float32, tag="psb")
                for n in range(2):
                    for kc in range(8):
                        nc.tensor.matmul(
                            ps[:, ts(n, 512)],
                            xT_sb[:, kc, ts(st, 128)],
                            w3_sb[:, kc, ds(pj * CH + n * 512, 512)],
                            start=(kc == 0), stop=(kc == 7))
                if pj == 1:
                    kvt_cur = kv_pool.tile([128, 2, CH], dt.bfloat16, tag="kvt")
                    nc.scalar.copy(kvt_cur[:, 0, :], ps[:])
                else:
                    nc.scalar.copy(kvt_cur[:, 1, :], ps[:])
                    kv_stores.append(nc.sync.dma_start(
                        kv_loc[ts(st, 128), :],
                        kvt_cur[:].rearrange("p a b -> p (a b)")))
        cc_i = nc.gpsimd.collective_compute(
            "AllGather", mybir.AluOpType.bypass,
            replica_groups=[list(range(NCORES))],
            ins=[kv_loc[:, :]], outs=[kv_full[:, :]])
        for stn in kv_stores:
            add_dep_helper(cc_i.ins, stn.ins, sync=True, reason="cc after kv stores")
        cc_insts = [cc_i]
        for st in range(NST):         # q after all k/v (overlaps AllGather)
            ps = ps_big.tile([128, CH], dt.float32, tag="psb")
            for n in range(2):
                for kc in range(8):
                    nc.tensor.matmul(
                        ps[:, ts(n, 512)],
                        xT_sb[:, kc, ts(st, 128)],
                        w3_sb[:, kc, ds(n * 512, 512)],
                        start=(kc == 0), stop=(kc == 7))
            nc.scalar.copy(q_sb[:, st, :], ps[:])
        wa.__exit__(None, None, None)
        gat = ctx.enter_context(tc.tile_pool(name="gat", bufs=5))
        big = ctx.enter_context(tc.tile_pool(name="big", bufs=2))
        small = ctx.enter_context(tc.tile_pool(name="small", bufs=3))
        atg_pool = ctx.enter_context(tc.tile_pool(name="atg", bufs=2))
        outp = ctx.enter_context(tc.tile_pool(name="outp", bufs=1))

        # ---- phase B: per-tile sparse attention, 2-stage software pipe ----
        NT = NTB if mode != "proj" else 0
        kvsel = {}      # tile -> gathered k|v rows [128, NCC, ROW]
        qrep = {}       # tile -> q replicated to 128 partitions
        e2 = {}         # tile -> exp(logits), pair-expanded
        r2rep = {}      # tile -> 1/den replicated to 128 partitions
        atg_cur = None

        def emit_gather(t):
            kvsel[t] = gat.tile([128, NCC, ROW], dt.bfloat16, tag="kvsel", name="kvsel")
            g = nc.gpsimd.dma_gather(
                out_ap=kvsel[t][:], in_ap=kv_full[:, :],
                idxs_ap=idx_sb[:, ds(t * 32, 32)],
                num_idxs=QT * K, num_idxs_reg=QT * K,
                elem_size=ROW, single_packet=False)
            for cci in cc_insts:
                add_dep_helper(g.ins, cci.ins, sync=True, reason="gather after cc")

        def emit_qrep(t):
            st, g16 = t // 8, t % 8
            qrep[t] = small.tile([128, CH], dt.bfloat16, tag="qrep", name="qrep")
            for n in range(2):
                psq = ps_q.tile([128, 512], dt.float32, tag="psq")
                nc.tensor.matmul(psq[:], selrep_sb[:, g16, :],
                                 q_sb[:, st, ts(n, 512)], start=True, stop=True)
                nc.scalar.copy(qrep[t][:, ts(n, 512)], psq[:])

        def emit_stage_a(t):
            # logits: t1 = q*k, halving-tree d-reduction (pure 2x TT ops)
            t1 = big.tile([128, NCC, CH], dt.bfloat16, tag="t1")
            k_ap = kvsel[t][:, :, 0:CH]
            k_ap2, q_ap2 = bass.broadcast_tensor_aps(
                k_ap, qrep[t][:].rearrange("p (o c) -> p o c", o=1))
            nc.vector.tensor_mul(t1[:], k_ap2, q_ap2)
            t1v = t1[:].rearrange("p c (h d) -> p (c h) d", d=D)
            nc.vector.tensor_add(t1v[:, :, 0:32], t1v[:, :, 0:32], t1v[:, :, 32:64])
            nc.vector.tensor_add(t1v[:, :, 0:16], t1v[:, :, 0:16], t1v[:, :, 16:32])
            nc.vector.tensor_add(t1v[:, :, 0:8], t1v[:, :, 0:8], t1v[:, :, 8:16])
            nc.vector.tensor_add(t1v[:, :, 0:4], t1v[:, :, 0:4], t1v[:, :, 4:8])
            nc.vector.tensor_add(t1v[:, :, 0:2], t1v[:, :, 0:2], t1v[:, :, 2:4])
            lgt = small.tile([128, 4 * NH], dt.float32, tag="lgt")
            lgtv = lgt[:].rearrange("p (g o) -> p g o", o=1)
            nc.vector.tensor_add(lgtv, t1v[:, :, 0:1], t1v[:, :, 1:2])
            nc.vector.tensor_add(lgt[:], lgt[:], gb_sb[:, t, :])

            # e2 = exp(logits), pair-expanded (ACT)
            e2[t] = small.tile([128, NCC, NH, 2], dt.bfloat16, tag="e2", name="e2")
            lgt4 = lgt[:].rearrange("p (c h o) -> p c h o", c=NCC, o=1)
            e2a, lgt4b = bass.broadcast_tensor_aps(e2[t][:], lgt4)
            nc.scalar.activation(e2a, lgt4b, EXP)

            # denominator on PE (both pair lanes kept)
            psd = ps_sm.tile([16, 2 * NH], dt.float32, tag="pss")
            for cc in range(NCC):
                nc.tensor.matmul(psd[:], s16_sb[:],
                                 e2[t][:, cc, :, :].rearrange("p h w -> p (h w)"),
                                 start=(cc == 0), stop=(cc == NCC - 1))
            return psd

        def emit_recip_rep(t, psd):
            # r16 = 1/den on 16 partitions, then replicate to 128 via PE
            r16 = small.tile([16, 2 * NH], dt.float32, tag="r16")
            nc.vector.reciprocal_approx_fast(r16[:], psd[:])
            psr = ps_sm.tile([128, 2 * NH], dt.float32, tag="pss")
            nc.tensor.matmul(psr[:], s16t_sb[:], r16[:], start=True, stop=True)
            r2rep[t] = small.tile([128, 1, NH, 2], dt.bfloat16, tag="r2rep", name="r2rep")
            nc.scalar.copy(r2rep[t][:].rearrange("p o h w -> p (o h w)"), psr[:])

        def emit_stage_b(t):
            nonlocal atg_cur
            st, g16 = t // 8, t % 8
            # e2n = e2 * (1/den) -- normalized weights, pair-expanded
            e2n = small.tile([128, NCC, NH, 2], dt.bfloat16, tag="e2n")
            e_in, r_in = bass.broadcast_tensor_aps(e2[t][:], r2rep[t][:])
            nc.vector.tensor_mul(e2n[:], e_in, r_in)
            # W = v * e2n (bcast over d, pair-expanded 2x)
            W = big.tile([128, NCC, CH], dt.bfloat16, tag="W")
            v_ap2, e_ap2 = bass.broadcast_tensor_aps(
                kvsel[t][:, :, CH:ROW].rearrange(
                    "p c (h dd w) -> p c h dd w", dd=32, w=2),
                e2n[:].rearrange("p c h (dd w) -> p c h dd w", dd=1, w=2))
            nc.vector.tensor_mul(
                W[:].rearrange("p c (h dd w) -> p c h dd w", dd=32, w=2),
                v_ap2, e_ap2)
            # AV flip: psAT[ch, q] = sum_p W[p, ch] s16[p, q]; constant
            # stationary, out free=16 -> 32 matmuls pipelining at ~30ns
            psat = ps_sm.tile([128, 8, QT], dt.float32, tag="pss")
            for chk in range(8):
                for cc in range(NCC):
                    nc.tensor.matmul(psat[:, chk, :],
                                     W[:, cc, ts(chk, 128)], s16_sb[:],
                                     start=(cc == 0), stop=(cc == NCC - 1))
            if g16 == 0:
                atg_cur = atg_pool.tile([128, 8, 128], dt.bfloat16, tag="atg")
            nc.scalar.copy(atg_cur[:, :, ds(QT * g16, QT)], psat[:])
            # o-proj per group of 8 tiles (128 query rows)
            if g16 == 7:
                psP = ps_big.tile([128, H], dt.float32, tag="psb")
                for n in range(2):
                    for chk in range(8):
                        nc.tensor.matmul(psP[:, ts(n, 512)], atg_cur[:, chk, :],
                                         wo_sb[:, chk, ts(n, 512)],
                                         start=(chk == 0), stop=(chk == 7))
                ot = outp.tile([128, H], dt.float32, tag="ot")
                nc.scalar.copy(ot[:], psP[:])
                nc.sync.dma_start(outd[ts(st, 128), :], ot[:])

        if NT > 0:
            emit_gather(0)
            emit_qrep(0)
        for i in range(NT):
            if i + 1 < NT:
                emit_gather(i + 1)
                emit_qrep(i + 1)
            psd = emit_stage_a(i)
            if i > 0:
                emit_stage_b(i - 1)
            emit_recip_rep(i, psd)
        if NT > 0:
            emit_stage_b(NT - 1)

    nc.compile()
    return nc


def prep_inputs(x, idx, valid, geo_bias, Wq, Wk, Wv, Wo, bo, epoch=1):
    """Host-side shard prep. Returns (in_maps, bo_f32)."""
    x = np.asarray(x)
    idx = np.asarray(idx)
    geo_bias = np.asarray(geo_bias)
    Wq, Wk, Wv, Wo = (np.asarray(w) for w in (Wq, Wk, Wv, Wo))
    bo = np.asarray(bo, dtype=np.float32)

    x2 = x.reshape(S, H)
    scale = np.float32(1.0 / np.sqrt(D))
    w3T = np.ascontiguousarray(
        np.concatenate([(Wq * scale).T, Wk.T, Wv.T], axis=1).astype(BF16))
    woT = np.ascontiguousarray(Wo.T.astype(BF16))
    s16 = np.zeros((128, 16), dtype=BF16)
    s16[np.arange(128), np.arange(128) % 16] = 1
    s16t = np.zeros((16, 128), dtype=np.float32)
    s16t[np.arange(128) % 16, np.arange(128)] = 1
    # selrep[r, g, m] = 1 iff r == 16 g + m%16  (q replication matrices)
    selrep = np.zeros((128, 8, 128), dtype=BF16)
    for g in range(8):
        m = np.arange(128)
        selrep[16 * g + m % 16, g, m] = 1
    selrep = np.ascontiguousarray(selrep.reshape(128, 8 * 128))

    in_maps = []
    for c in range(NCORES):
        rb = c * SC
        xTc = np.ascontiguousarray(x2[rb:rb + SC].T.astype(BF16))

        # gather indices: tile t, pos = j*16 + q -> idx[rb + t*16 + q, j]
        idxc = np.empty((16, NTB * 32), dtype=np.int16)
        for t in range(NTB):
            blk = idx[rb + t * QT: rb + (t + 1) * QT, :]      # [16 q, 32 j]
            lin = np.asarray(blk.T.reshape(-1))                # pos = j*16+q
            idxc[:, t * 32:(t + 1) * 32] = lin.reshape(32, 16).T.astype(np.int16)
        idxc = np.ascontiguousarray(np.tile(idxc, (8, 1)))

        # geo bias: gb[p=(b,qq), t, cc*16+h] = geo_bias[h, rb+t*16+qq, cc*8+b]
        gg = geo_bias[:, rb:rb + SC, :]                        # [h, 512, j]
        g2 = gg.reshape(NH, NTB, QT, NCC, 8)                   # [h, t, qq, cc, b]
        gbt = g2.transpose(4, 2, 1, 3, 0).reshape(128, NTB * 4 * NH)
        gbt = np.ascontiguousarray(gbt, dtype=np.float32)

        in_maps.append({
            "xT": xTc,
            "w3T": w3T,
            "woT": woT,
            "gb": gbt,
            "idx16": idxc,
            "s16": s16,
            "s16t": s16t,
            "selrep": selrep,
        })
    return in_maps, bo


_epoch = 0


def kernel(x, idx, valid, geo_bias, Wq, Wk, Wv, Wo, bo):
    global _nc_cache, _epoch
    from concourse.bass_utils import run_bass_kernel_spmd

    if _nc_cache is None:
        _nc_cache = build_nc()
    nc = _nc_cache

    _epoch += 1
    in_maps, bo_f32 = prep_inputs(x, idx, valid, geo_bias, Wq, Wk, Wv, Wo, bo,
                                  epoch=_epoch)
    res = run_bass_kernel_spmd(nc, in_maps, core_ids=list(range(NCORES)),
                               trace=bool(int(os.environ.get("KTRACE", "0"))))
    out = np.concatenate([r["out"] for r in res.results], axis=0)
    out = out + bo_f32[None, :]
    if res.exec_time_ns is not None:
        kernel.last_exec_time_ns = res.exec_time_ns
    kernel.last_results = res
    return out.reshape(1, S, H).astype(np.float32)


# revision 22
# speedup vs baseline: 1.1370x; 1.0230x over previous
"""Sparse-attention Trainium2 kernel (8 NeuronCores, sequence-parallel).

Problem (hardcoded): B=1, S=4096, H=1024, NH=16, D=64, K=32.

Sharding: fully sequence-parallel. Core c owns query rows [512c, 512c+512).
It computes q/k/v for its own rows against the FULL weight matrices,
publishes its k|v rows via an 8-way AllGather, then dma_gathers per-query
k/v rows for ALL 16 heads at once (4 KB/descriptor), computes the sparse
attention for its rows and the o-projection. Host concatenates row slices.

Phase-B dataflow (v2, rebalanced vs the 494us baseline):
- q is kept in SBUF from phase A and replicated 16->128 partitions per tile
  by a PE matmul against a static selection matrix (selrep), killing the
  per-tile SWDGE qrep gather (Pool + DMA traffic).
- softmax normalization moved BEFORE the AV product: r16 = 1/den on 16
  partitions is partition-replicated to 128 via a PE matmul (s16t), then
  e2n = e2 * r2rep on DVE (free 128) replaces the [16,1024] A-normalize.
- AV "flip": psAT[ch,q] = sum_p W[p,ch] * s16[p,q] -- moving=W chunk,
  stationary=s16 (constant!), out free=16. 32 small matmuls pipeline at
  ~30ns spacing, replacing the old 8x630ns psA + 8 PE transposes; the
  PSUM result IS A^T so it copies straight into the o-proj group buffer.
- two-stage software pipeline: per iteration emit gather(i+1)/qrep(i+1),
  logits+exp+den+recip for tile i, and e2n/W/psAT for tile i-1, so the
  DVE never head-of-line blocks on the exp->den->recip round trip.

Measured: 454us (vs 489us baseline), rel err 5.6e-3. Phase A 0-75us
(PE-bound projections), AllGather 75-155us (ALL engines idle), phase B
160-450us at ~9.0us/tile with DVE 100% busy (~8.1us/tile: t1 mul 2.27 +
tree 2.82 + W mul 2.29 + e2n/recip/small). DVE does ~2.2 passes over the
64MB/core of gathered kv at ~490GB/s -- that is the structural floor of
this dataflow. PE after the flip ~4.2; DMA ~6.0; Pool = descgen only.

Lessons measured on HW (do NOT retry without new information):
- remote_dma_broadcast peer-to-peer kv exchange (to kill the ~80us
  collective window): NEFF dies with INTERNAL error even with all sem
  waits removed -- the remote-DMA/SWDGE-broadcast transport appears
  nonfunctional under this axon-tunneled runtime. The full epoch-safe
  handshake design is in git-history/transcript if transport ever works.
- num_swdge_queues=2 with gathers alternating queues: INTERNAL error.
- Splitting xT/w3 into per-kc tiles for earlier matmul start: 462us
  (slightly WORSE than monolithic, possibly variance; reverted).
- Prefetching gathers 2 iterations ahead (gat bufs 6): 504us, much
  WORSE -- the deeper in-flight gather queue delays the tile at the
  head of the DMA-engine pool. Keep prefetch depth 1.
- 4-way chunked AllGather: worse (~26us fixed per chunk, prior session).
- Pool-engine tensor ops to offload DVE: impossible -- InstTensorTensor
  is not in the mlp/attnmlp GPSIMD libraries that dma_gather needs, so
  alternating them would thrash mid-kernel library reloads every tile.
- fp8 kv: DVE 2x mode requires 2-byte dtypes, so fp8 operands halve DVE
  throughput -- costs more than the DMA it saves (prior session: ACT
  upcast also too slow). kvsel gathers need single_packet=False.
- PE logits (k^T transpose-gather) always land as [pos, q'] outer
  products needing a per-partition diagonal read no AP can express.
"""

import os
from contextlib import ExitStack

import numpy as np
import ml_dtypes

S, H, NH, D, K = 4096, 1024, 16, 64, 32
NCORES = 8
SC = S // NCORES            # 512 rows per core
QT = 16                     # queries per attention tile
NTB = SC // QT              # 32 attention tiles per core
NST = SC // 128             # 4 projection s-tiles per core
CH = NH * D                 # 1024 kv channels per tensor
ROW = 2 * CH                # 2048 bf16 elems per kv row (4 KB)
NCC = K // 8                # 4 slot chunks per tile
BF16 = ml_dtypes.bfloat16

_nc_cache = None


def build_nc(mode="full"):
    import concourse.bass as bass
    import concourse.mybir as mybir
    import concourse.tile as tile
    from concourse import bacc
    from concourse.tile_rust import add_dep_helper
    from concourse.bass import ts, ds

    dt = mybir.dt
    nc = bacc.Bacc("TRN2", target_bir_lowering=False, debug=False,
                   num_devices=NCORES)

    xT = nc.dram_tensor("xT", [H, SC], dt.bfloat16, kind="ExternalInput")
    w3T = nc.dram_tensor("w3T", [H, 3 * CH], dt.bfloat16, kind="ExternalInput")
    woT = nc.dram_tensor("woT", [CH, H], dt.bfloat16, kind="ExternalInput")
    gb = nc.dram_tensor("gb", [128, NTB * 4 * NH], dt.float32, kind="ExternalInput")
    idx16 = nc.dram_tensor("idx16", [128, NTB * (QT * K // 16)], dt.int16,
                           kind="ExternalInput")
    s16d = nc.dram_tensor("s16", [128, 16], dt.bfloat16, kind="ExternalInput")
    s16td = nc.dram_tensor("s16t", [16, 128], dt.float32, kind="ExternalInput")
    selrepd = nc.dram_tensor("selrep", [128, 8 * 128], dt.bfloat16,
                             kind="ExternalInput")
    outd = nc.dram_tensor("out", [SC, H], dt.float32, kind="ExternalOutput")
    kv_loc = nc.dram_tensor("kv_loc", [SC, ROW], dt.bfloat16, kind="Internal")
    kv_full = nc.dram_tensor("kv_full", [S, ROW], dt.bfloat16, kind="Internal",
                             addr_space="Shared")

    EXP = mybir.ActivationFunctionType.Exp

    with ExitStack() as ctx:
        tc = ctx.enter_context(tile.TileContext(nc))
        const = ctx.enter_context(tc.tile_pool(name="const", bufs=1))

        kv_pool = ctx.enter_context(tc.tile_pool(name="kvout", bufs=2))
        ps_big = ctx.enter_context(tc.tile_pool(name="ps_big", bufs=2, space="PSUM"))
        ps_q = ctx.enter_context(tc.tile_pool(name="ps_q", bufs=2, space="PSUM"))
        ps_sm = ctx.enter_context(tc.tile_pool(name="ps_sm", bufs=2, space="PSUM"))

        # ---- phase-A weights first: the kv matmuls gate the collective ----
        wa = tc.tile_pool(name="wa", bufs=1)
        wap = wa.__enter__()
        xT_sb = wap.tile([128, 8, SC], dt.bfloat16)           # 1 MB, phase A only
        for kc in range(8):
            nc.sync.dma_start(xT_sb[:, kc, :], xT[ts(kc, 128), :])
        w3_sb = wap.tile([128, 8, 3 * CH], dt.bfloat16)       # 6 MB, phase A only
        for kc in range(8):
            nc.sync.dma_start(w3_sb[:, kc, ds(CH, 2 * CH)],
                              w3T[ts(kc, 128), ds(CH, 2 * CH)])
        for kc in range(8):
            nc.sync.dma_start(w3_sb[:, kc, ds(0, CH)], w3T[ts(kc, 128), ds(0, CH)])

        # ---- small resident tensors ----
        idx_sb = const.tile([128, NTB * 32], dt.int16)        # 0.25 MB
        nc.sync.dma_start(idx_sb[:], idx16[:, :])
        gb_sb = const.tile([128, NTB, 4 * NH], dt.float32)    # 1 MB
        nc.sync.dma_start(gb_sb[:], gb[:, :])
        s16_sb = const.tile([128, 16], dt.bfloat16)
        nc.sync.dma_start(s16_sb[:], s16d[:, :])
        s16t_sb = const.tile([16, 128], dt.float32)
        nc.sync.dma_start(s16t_sb[:], s16td[:, :])
        selrep_sb = const.tile([128, 8, 128], dt.bfloat16)
        nc.sync.dma_start(selrep_sb[:],
                          selrepd[:, :].rearrange("p (g m) -> p g m", g=8))
        wo_sb = const.tile([128, 8, H], dt.bfloat16)          # 2 MB
        for chn in range(8):
            nc.sync.dma_start(wo_sb[:, chn, :], woT[ts(chn, 128), :])
        q_sb = const.tile([128, NST, CH], dt.bfloat16)        # 1 MB, lives all of B

        # ---- phase A: k/v first (AllGather can start early), then q ----
        kv_stores = []
        for st in range(NST):
            kvt_cur = None
            for pj in (1, 2):         # k then v
                ps = ps_big.tile([128, CH], dt.float32, tag="psb")
                for n in range(2):
                    for kc in range(8):
                        nc.tensor.matmul(
                            ps[:, ts(n, 512)],
                            xT_sb[:, kc, ts(st, 128)],
                            w3_sb[:, kc, ds(pj * CH + n * 512, 512)],
                            start=(kc == 0), stop=(kc == 7))
                if pj == 1:
                    kvt_cur = kv_pool.tile([128, 2, CH], dt.bfloat16, tag="kvt")
                    nc.scalar.copy(kvt_cur[:, 0, :], ps[:])
                else:
                    nc.scalar.copy(kvt_cur[:, 1, :], ps[:])
                    kv_stores.append(nc.sync.dma_start(
                        kv_loc[ts(st, 128), :],
                        kvt_cur[:].rearrange("p a b -> p (a b)")))
        cc_i = nc.gpsimd.collective_compute(
            "AllGather", mybir.AluOpType.bypass,
            replica_groups=[list(range(NCORES))],
            ins=[kv_loc[:, :]], outs=[kv_full[:, :]])
        for stn in kv_stores:
            add_dep_helper(cc_i.ins, stn.ins, sync=True, reason="cc after kv stores")
        cc_insts = [cc_i]
        for st in range(NST):         # q after all k/v (overlaps AllGather)
            ps = ps_big.tile([128, CH], dt.float32, tag="psb")
            for n in range(2):
                for kc in range(8):
                    nc.tensor.matmul(
                        ps[:, ts(n, 512)],
                        xT_sb[:, kc, ts(st, 128)],
                        w3_sb[:, kc, ds(n * 512, 512)],
                        start=(kc == 0), stop=(kc == 7))
            nc.scalar.copy(q_sb[:, st, :], ps[:])
        wa.__exit__(None, None, None)
        gat = ctx.enter_context(tc.tile_pool(name="gat", bufs=5))
        big = ctx.enter_context(tc.tile_pool(name="big", bufs=2))
        small = ctx.enter_context(tc.tile_pool(name="small", bufs=3))
        atg_pool = ctx.enter_context(tc.tile_pool(name="atg", bufs=2))
        outp = ctx.enter_context(tc.tile_pool(name="outp", bufs=1))

        # ---- phase B: per-tile sparse attention, 2-stage software pipe ----
        NT = NTB if mode != "proj" else 0
        kvsel = {}      # tile -> gathered k|v rows [128, NCC, ROW]
        qrep = {}       # tile -> q replicated to 128 partitions
        e2 = {}         # tile -> exp(logits), pair-expanded
        r2rep = {}      # tile -> 1/den replicated to 128 partitions
        atg_cur = None

        def emit_gather(t):
            kvsel[t] = gat.tile([128, NCC, ROW], dt.bfloat16, tag="kvsel", name="kvsel")
            g = nc.gpsimd.dma_gather(
                out_ap=kvsel[t][:], in_ap=kv_full[:, :],
                idxs_ap=idx_sb[:, ds(t * 32, 32)],
                num_idxs=QT * K, num_idxs_reg=QT * K,
                elem_size=ROW, single_packet=False)
            for cci in cc_insts:
                add_dep_helper(g.ins, cci.ins, sync=True, reason="gather after cc")

        def emit_qrep(t):
            st, g16 = t // 8, t % 8
            qrep[t] = small.tile([128, CH], dt.bfloat16, tag="qrep", name="qrep")
            for n in range(2):
                psq = ps_q.tile([128, 512], dt.float32, tag="psq")
                nc.tensor.matmul(psq[:], selrep_sb[:, g16, :],
                                 q_sb[:, st, ts(n, 512)], start=True, stop=True)
                nc.scalar.copy(qrep[t][:, ts(n, 512)], psq[:])

        def emit_stage_a(t):
            # logits: t1 = q*k, halving-tree d-reduction (pure 2x TT ops)
            t1 = big.tile([128, NCC, CH], dt.bfloat16, tag="t1")
            k_ap = kvsel[t][:, :, 0:CH]
            k_ap2, q_ap2 = bass.broadcast_tensor_aps(
                k_ap, qrep[t][:].rearrange("p (o c) -> p o c", o=1))
            nc.vector.tensor_mul(t1[:], k_ap2, q_ap2)
            t1v = t1[:].rearrange("p c (h d) -> p (c h) d", d=D)
            nc.vector.tensor_add(t1v[:, :, 0:32], t1v[:, :, 0:32], t1v[:, :, 32:64])
            nc.vector.tensor_add(t1v[:, :, 0:16], t1v[:, :, 0:16], t1v[:, :, 16:32])
            nc.vector.tensor_add(t1v[:, :, 0:8], t1v[:, :, 0:8], t1v[:, :, 8:16])
            nc.vector.tensor_add(t1v[:, :, 0:4], t1v[:, :, 0:4], t1v[:, :, 4:8])
            nc.vector.tensor_add(t1v[:, :, 0:2], t1v[:, :, 0:2], t1v[:, :, 2:4])
            lgt = small.tile([128, 4 * NH], dt.float32, tag="lgt")
            lgtv = lgt[:].rearrange("p (g o) -> p g o", o=1)
            nc.vector.tensor_add(lgtv, t1v[:, :, 0:1], t1v[:, :, 1:2])
            nc.vector.tensor_add(lgt[:], lgt[:], gb_sb[:, t, :])

            # e2 = exp(logits), pair-expanded (ACT)
            e2[t] = small.tile([128, NCC, NH, 2], dt.bfloat16, tag="e2", name="e2")
            lgt4 = lgt[:].rearrange("p (c h o) -> p c h o", c=NCC, o=1)
            e2a, lgt4b = bass.broadcast_tensor_aps(e2[t][:], lgt4)
            nc.scalar.activation(e2a, lgt4b, EXP)

            # denominator on PE (both pair lanes kept)
            psd = ps_sm.tile([16, 2 * NH], dt.float32, tag="pss")
            for cc in range(NCC):
                nc.tensor.matmul(psd[:], s16_sb[:],
                                 e2[t][:, cc, :, :].rearrange("p h w -> p (h w)"),
                                 start=(cc == 0), stop=(cc == NCC - 1))
            return psd

        def emit_recip_rep(t, psd):
            # r16 = 1/den on 16 partitions, then replicate to 128 via PE
            r16 = small.tile([16, 2 * NH], dt.float32, tag="r16")
            nc.vector.reciprocal_approx_fast(r16[:], psd[:])
            psr = ps_sm.tile([128, 2 * NH], dt.float32, tag="pss")
            nc.tensor.matmul(psr[:], s16t_sb[:], r16[:], start=True, stop=True)
            r2rep[t] = small.tile([128, 1, NH, 2], dt.bfloat16, tag="r2rep", name="r2rep")
            nc.scalar.copy(r2rep[t][:].rearrange("p o h w -> p (o h w)"), psr[:])

        def emit_stage_b_dve(t):
            # e2n = e2 * (1/den) -- normalized weights, pair-expanded
            e2n = small.tile([128, NCC, NH, 2], dt.bfloat16, tag="e2n")
            e_in, r_in = bass.broadcast_tensor_aps(e2[t][:], r2rep[t][:])
            nc.vector.tensor_mul(e2n[:], e_in, r_in)
            # W = v * e2n (bcast over d, pair-expanded 2x)
            W = big.tile([128, NCC, CH], dt.bfloat16, tag="W", name="W")
            v_ap2, e_ap2 = bass.broadcast_tensor_aps(
                kvsel[t][:, :, CH:ROW].rearrange(
                    "p c (h dd w) -> p c h dd w", dd=32, w=2),
                e2n[:].rearrange("p c h (dd w) -> p c h dd w", dd=1, w=2))
            nc.vector.tensor_mul(
                W[:].rearrange("p c (h dd w) -> p c h dd w", dd=32, w=2),
                v_ap2, e_ap2)
            return W

        def emit_stage_b_pe(t, W):
            nonlocal atg_cur
            st, g16 = t // 8, t % 8
            # AV flip: psAT[ch, q] = sum_p W[p, ch] s16[p, q]; constant
            # stationary, out free=16 -> 32 matmuls pipelining at ~30ns
            psat = ps_sm.tile([128, 8, QT], dt.float32, tag="pss")
            for chk in range(8):
                for cc in range(NCC):
                    nc.tensor.matmul(psat[:, chk, :],
                                     W[:, cc, ts(chk, 128)], s16_sb[:],
                                     start=(cc == 0), stop=(cc == NCC - 1))
            if g16 == 0:
                atg_cur = atg_pool.tile([128, 8, 128], dt.bfloat16, tag="atg",
                                        name="atg")
            nc.scalar.copy(atg_cur[:, :, ds(QT * g16, QT)], psat[:])

        oproj_q = []

        def emit_oproj_slice():
            # o-proj for a completed 8-tile group, spread over 4 iterations
            # (4 matmuls each) so the in-order PE queue never bursts ~10us
            # and delays the next tiles' psd/psr (which feed the DVE).
            if not oproj_q:
                return
            st, atg_t, psP, k = oproj_q[0]
            for chk in range(4 * k, 4 * k + 4):
                n, c8 = chk // 8, chk % 8
                nc.tensor.matmul(psP[:, ts(n, 512)], atg_t[:, c8, :],
                                 wo_sb[:, c8, ts(n, 512)],
                                 start=(c8 == 0), stop=(c8 == 7))
            if k == 3:
                oproj_q.pop(0)
                ot = outp.tile([128, H], dt.float32, tag="ot", name="ot")
                nc.scalar.copy(ot[:], psP[:])
                nc.sync.dma_start(outd[ts(st, 128), :], ot[:])
            else:
                oproj_q[0] = (st, atg_t, psP, k + 1)

        if NT > 0:
            emit_gather(0)
            emit_qrep(0)
        for i in range(NT):
            if i + 1 < NT:
                emit_gather(i + 1)
                emit_qrep(i + 1)
            psd = emit_stage_a(i)
            W_prev = emit_stage_b_dve(i - 1) if i > 0 else None
            # recip for tile i: exp(i)/psd(i) complete during e2n/W(i-1),
            # and psr(i) runs on PE BEFORE psat(i-1) so r2rep(i) is ready
            # long before e2n(i) next iteration
            emit_recip_rep(i, psd)
            if i > 0:
                emit_stage_b_pe(i - 1, W_prev)
                if (i - 1) % 8 == 7:
                    psP = ps_big.tile([128, H], dt.float32, tag="psb",
                                      name="psP")
                    oproj_q.append(((i - 1) // 8, atg_cur, psP, 0))
            emit_oproj_slice()
        if NT > 0:
            W_last = emit_stage_b_dve(NT - 1)
            emit_stage_b_pe(NT - 1, W_last)
            psP = ps_big.tile([128, H], dt.float32, tag="psb", name="psP")
            oproj_q.append(((NT - 1) // 8, atg_cur, psP, 0))
            while oproj_q:
                emit_oproj_slice()

    nc.compile()
    return nc


def prep_inputs(x, idx, valid, geo_bias, Wq, Wk, Wv, Wo, bo, epoch=1):
    """Host-side shard prep. Returns (in_maps, bo_f32)."""
    x = np.asarray(x)
    idx = np.asarray(idx)
    geo_bias = np.asarray(geo_bias)
    Wq, Wk, Wv, Wo = (np.asarray(w) for w in (Wq, Wk, Wv, Wo))
    bo = np.asarray(bo, dtype=np.float32)

    x2 = x.reshape(S, H)
    scale = np.float32(1.0 / np.sqrt(D))
    w3T = np.ascontiguousarray(
        np.concatenate([(Wq * scale).T, Wk.T, Wv.T], axis=1).astype(BF16))
    woT = np.ascontiguousarray(Wo.T.astype(BF16))
    s16 = np.zeros((128, 16), dtype=BF16)
    s16[np.arange(128), np.arange(128) % 16] = 1
    s16t = np.zeros((16, 128), dtype=np.float32)
    s16t[np.arange(128) % 16, np.arange(128)] = 1
    # selrep[r, g, m] = 1 iff r == 16 g + m%16  (q replication matrices)
    selrep = np.zeros((128, 8, 128), dtype=BF16)
    for g in range(8):
        m = np.arange(128)
        selrep[16 * g + m % 16, g, m] = 1
    selrep = np.ascontiguousarray(selrep.reshape(128, 8 * 128))

    in_maps = []
    for c in range(NCORES):
        rb = c * SC
        xTc = np.ascontiguousarray(x2[rb:rb + SC].T.astype(BF16))

        # gather indices: tile t, pos = j*16 + q -> idx[rb + t*16 + q, j]
        idxc = np.empty((16, NTB * 32), dtype=np.int16)
        for t in range(NTB):
            blk = idx[rb + t * QT: rb + (t + 1) * QT, :]      # [16 q, 32 j]
            lin = np.asarray(blk.T.reshape(-1))                # pos = j*16+q
            idxc[:, t * 32:(t + 1) * 32] = lin.reshape(32, 16).T.astype(np.int16)
        idxc = np.ascontiguousarray(np.tile(idxc, (8, 1)))

        # geo bias: gb[p=(b,qq), t, cc*16+h] = geo_bias[h, rb+t*16+qq, cc*8+b]
        gg = geo_bias[:, rb:rb + SC, :]                        # [h, 512, j]
        g2 = gg.reshape(NH, NTB, QT, NCC, 8)                   # [h, t, qq, cc, b]
        gbt = g2.transpose(4, 2, 1, 3, 0).reshape(128, NTB * 4 * NH)
        gbt = np.ascontiguousarray(gbt, dtype=np.float32)

        in_maps.append({
            "xT": xTc,
            "w3T": w3T,
            "woT": woT,
            "gb": gbt,
            "idx16": idxc,
            "s16": s16,
            "s16t": s16t,
            "selrep": selrep,
        })
    return in_maps, bo


_epoch = 0


def kernel(x, idx, valid, geo_bias, Wq, Wk, Wv, Wo, bo):
    global _nc_cache, _epoch
    from concourse.bass_utils import run_bass_kernel_spmd

    if _nc_cache is None:
        _nc_cache = build_nc()
    nc = _nc_cache

    _epoch += 1
    in_maps, bo_f32 = prep_inputs(x, idx, valid, geo_bias, Wq, Wk, Wv, Wo, bo,
                                  epoch=_epoch)
    res = run_bass_kernel_spmd(nc, in_maps, core_ids=list(range(NCORES)),
                               trace=bool(int(os.environ.get("KTRACE", "0"))))
    out = np.concatenate([r["out"] for r in res.results], axis=0)
    out = out + bo_f32[None, :]
    if res.exec_time_ns is not None:
        kernel.last_exec_time_ns = res.exec_time_ns
    kernel.last_results = res
    return out.reshape(1, S, H).astype(np.float32)


# revision 24
# speedup vs baseline: 1.1452x; 1.0072x over previous
"""Sparse-attention Trainium2 kernel (8 NeuronCores, sequence-parallel).

Problem (hardcoded): B=1, S=4096, H=1024, NH=16, D=64, K=32.

Sharding: fully sequence-parallel. Core c owns query rows [512c, 512c+512).
It computes q/k/v for its own rows against the FULL weight matrices,
publishes its k|v rows via an 8-way AllGather, then dma_gathers per-query
k/v rows for ALL 16 heads at once (4 KB/descriptor), computes the sparse
attention for its rows and the o-projection. Host concatenates row slices.

Phase-B dataflow (v2, rebalanced vs the 494us baseline):
- q is kept in SBUF from phase A and replicated 16->128 partitions per tile
  by a PE matmul against a static selection matrix (selrep), killing the
  per-tile SWDGE qrep gather (Pool + DMA traffic).
- softmax normalization moved BEFORE the AV product: r16 = 1/den on 16
  partitions is partition-replicated to 128 via a PE matmul (s16t), then
  e2n = e2 * r2rep on DVE (free 128) replaces the [16,1024] A-normalize.
- AV "flip": psAT[ch,q] = sum_p W[p,ch] * s16[p,q] -- moving=W chunk,
  stationary=s16 (constant!), out free=16. 32 small matmuls pipeline at
  ~30ns spacing, replacing the old 8x630ns psA + 8 PE transposes; the
  PSUM result IS A^T so it copies straight into the o-proj group buffer.
- two-stage software pipeline: per iteration emit gather(i+1)/qrep(i+1),
  logits+exp+den+recip for tile i, and e2n/W/psAT for tile i-1, so the
  DVE never head-of-line blocks on the exp->den->recip round trip.

Measured: 454us (vs 489us baseline), rel err 5.6e-3. Phase A 0-75us
(PE-bound projections), AllGather 75-155us (ALL engines idle), phase B
160-450us at ~9.0us/tile with DVE 100% busy (~8.1us/tile: t1 mul 2.27 +
tree 2.82 + W mul 2.29 + e2n/recip/small). DVE does ~2.2 passes over the
64MB/core of gathered kv at ~490GB/s -- that is the structural floor of
this dataflow. PE after the flip ~4.2; DMA ~6.0; Pool = descgen only.

Lessons measured on HW (do NOT retry without new information):
- remote_dma_broadcast peer-to-peer kv exchange (to kill the ~80us
  collective window): NEFF dies with INTERNAL error even with all sem
  waits removed -- the remote-DMA/SWDGE-broadcast transport appears
  nonfunctional under this axon-tunneled runtime. The full epoch-safe
  handshake design is in git-history/transcript if transport ever works.
- num_swdge_queues=2 with gathers alternating queues: INTERNAL error.
- Splitting xT/w3 into per-kc tiles for earlier matmul start: 462us
  (slightly WORSE than monolithic, possibly variance; reverted).
- Prefetching gathers 2 iterations ahead (gat bufs 6): 504us, much
  WORSE -- the deeper in-flight gather queue delays the tile at the
  head of the DMA-engine pool. Keep prefetch depth 1.
- 4-way chunked AllGather: worse (~26us fixed per chunk, prior session).
- Pool-engine tensor ops to offload DVE: impossible -- InstTensorTensor
  is not in the mlp/attnmlp GPSIMD libraries that dma_gather needs, so
  alternating them would thrash mid-kernel library reloads every tile.
- fp8 kv: DVE 2x mode requires 2-byte dtypes, so fp8 operands halve DVE
  throughput -- costs more than the DMA it saves (prior session: ACT
  upcast also too slow). kvsel gathers need single_packet=False.
- PE logits (k^T transpose-gather) always land as [pos, q'] outer
  products needing a per-partition diagonal read no AP can express.
"""

import os
from contextlib import ExitStack

import numpy as np
import ml_dtypes

S, H, NH, D, K = 4096, 1024, 16, 64, 32
NCORES = 8
SC = S // NCORES            # 512 rows per core
QT = 16                     # queries per attention tile
NTB = SC // QT              # 32 attention tiles per core
NST = SC // 128             # 4 projection s-tiles per core
CH = NH * D                 # 1024 kv channels per tensor
ROW = 2 * CH                # 2048 bf16 elems per kv row (4 KB)
NCC = K // 8                # 4 slot chunks per tile
BF16 = ml_dtypes.bfloat16

_nc_cache = None


def build_nc(mode="full"):
    import concourse.bass as bass
    import concourse.mybir as mybir
    import concourse.tile as tile
    from concourse import bacc
    from concourse.tile_rust import add_dep_helper
    from concourse.bass import ts, ds

    dt = mybir.dt
    nc = bacc.Bacc("TRN2", target_bir_lowering=False, debug=False,
                   num_devices=NCORES)

    xT = nc.dram_tensor("xT", [H, SC], dt.bfloat16, kind="ExternalInput")
    w3T = nc.dram_tensor("w3T", [H, 3 * CH], dt.bfloat16, kind="ExternalInput")
    woT = nc.dram_tensor("woT", [CH, H], dt.bfloat16, kind="ExternalInput")
    gb = nc.dram_tensor("gb", [128, NTB * 4 * NH], dt.float32, kind="ExternalInput")
    idx16 = nc.dram_tensor("idx16", [128, NTB * (QT * K // 16)], dt.int16,
                           kind="ExternalInput")
    s16d = nc.dram_tensor("s16", [128, 16], dt.bfloat16, kind="ExternalInput")
    s16td = nc.dram_tensor("s16t", [16, 128], dt.float32, kind="ExternalInput")
    selrepd = nc.dram_tensor("selrep", [128, 8 * 128], dt.bfloat16,
                             kind="ExternalInput")
    outd = nc.dram_tensor("out", [SC, H], dt.float32, kind="ExternalOutput")
    kv_loc = nc.dram_tensor("kv_loc", [SC, ROW], dt.bfloat16, kind="Internal")
    kv_full = nc.dram_tensor("kv_full", [S, ROW], dt.bfloat16, kind="Internal",
                             addr_space="Shared")

    EXP = mybir.ActivationFunctionType.Exp

    with ExitStack() as ctx:
        tc = ctx.enter_context(tile.TileContext(nc))
        const = ctx.enter_context(tc.tile_pool(name="const", bufs=1))

        kv_pool = ctx.enter_context(tc.tile_pool(name="kvout", bufs=2))
        ps_big = ctx.enter_context(tc.tile_pool(name="ps_big", bufs=2, space="PSUM"))
        ps_q = ctx.enter_context(tc.tile_pool(name="ps_q", bufs=1, space="PSUM"))
        ps_sm = ctx.enter_context(tc.tile_pool(name="ps_sm", bufs=3, space="PSUM"))

        # ---- phase-A weights first: the kv matmuls gate the collective ----
        wa = tc.tile_pool(name="wa", bufs=1)
        wap = wa.__enter__()
        xT_sb = wap.tile([128, 8, SC], dt.bfloat16)           # 1 MB, phase A only
        for kc in range(8):
            nc.sync.dma_start(xT_sb[:, kc, :], xT[ts(kc, 128), :])
        w3_sb = wap.tile([128, 8, 3 * CH], dt.bfloat16)       # 6 MB, phase A only
        for kc in range(8):
            nc.sync.dma_start(w3_sb[:, kc, ds(CH, 2 * CH)],
                              w3T[ts(kc, 128), ds(CH, 2 * CH)])
        for kc in range(8):
            nc.sync.dma_start(w3_sb[:, kc, ds(0, CH)], w3T[ts(kc, 128), ds(0, CH)])

        # ---- small resident tensors ----
        idx_sb = const.tile([128, NTB * 32], dt.int16)        # 0.25 MB
        nc.sync.dma_start(idx_sb[:], idx16[:, :])
        gb_sb = const.tile([128, NTB, 4 * NH], dt.float32)    # 1 MB
        nc.sync.dma_start(gb_sb[:], gb[:, :])
        s16_sb = const.tile([128, 16], dt.bfloat16)
        nc.sync.dma_start(s16_sb[:], s16d[:, :])
        s16t_sb = const.tile([16, 128], dt.float32)
        nc.sync.dma_start(s16t_sb[:], s16td[:, :])
        selrep_sb = const.tile([128, 8, 128], dt.bfloat16)
        nc.sync.dma_start(selrep_sb[:],
                          selrepd[:, :].rearrange("p (g m) -> p g m", g=8))
        wo_sb = const.tile([128, 8, H], dt.bfloat16)          # 2 MB
        for chn in range(8):
            nc.sync.dma_start(wo_sb[:, chn, :], woT[ts(chn, 128), :])
        q_sb = const.tile([128, NST, CH], dt.bfloat16)        # 1 MB, lives all of B

        # ---- phase A: k/v first (AllGather can start early), then q ----
        kv_stores = []
        for st in range(NST):
            kvt_cur = None
            for pj in (1, 2):         # k then v
                ps = ps_big.tile([128, CH], dt.float32, tag="psb")
                for n in range(2):
                    for kc in range(8):
                        nc.tensor.matmul(
                            ps[:, ts(n, 512)],
                            xT_sb[:, kc, ts(st, 128)],
                            w3_sb[:, kc, ds(pj * CH + n * 512, 512)],
                            start=(kc == 0), stop=(kc == 7))
                if pj == 1:
                    kvt_cur = kv_pool.tile([128, 2, CH], dt.bfloat16, tag="kvt")
                    nc.scalar.copy(kvt_cur[:, 0, :], ps[:])
                else:
                    nc.scalar.copy(kvt_cur[:, 1, :], ps[:])
                    kv_stores.append(nc.sync.dma_start(
                        kv_loc[ts(st, 128), :],
                        kvt_cur[:].rearrange("p a b -> p (a b)")))
        cc_i = nc.gpsimd.collective_compute(
            "AllGather", mybir.AluOpType.bypass,
            replica_groups=[list(range(NCORES))],
            ins=[kv_loc[:, :]], outs=[kv_full[:, :]])
        for stn in kv_stores:
            add_dep_helper(cc_i.ins, stn.ins, sync=True, reason="cc after kv stores")
        cc_insts = [cc_i]
        for st in range(NST):         # q after all k/v (overlaps AllGather)
            ps = ps_big.tile([128, CH], dt.float32, tag="psb")
            for n in range(2):
                for kc in range(8):
                    nc.tensor.matmul(
                        ps[:, ts(n, 512)],
                        xT_sb[:, kc, ts(st, 128)],
                        w3_sb[:, kc, ds(n * 512, 512)],
                        start=(kc == 0), stop=(kc == 7))
            nc.scalar.copy(q_sb[:, st, :], ps[:])
        wa.__exit__(None, None, None)
        gat = ctx.enter_context(tc.tile_pool(name="gat", bufs=6))
        big = ctx.enter_context(tc.tile_pool(name="big", bufs=2))
        small = ctx.enter_context(tc.tile_pool(name="small", bufs=3))
        atg_pool = ctx.enter_context(tc.tile_pool(name="atg", bufs=2))
        outp = ctx.enter_context(tc.tile_pool(name="outp", bufs=1))

        # ---- phase B: per-tile sparse attention, 2-stage software pipe ----
        NT = NTB if mode != "proj" else 0
        kvsel = {}      # tile -> gathered k|v rows [128, NCC, ROW]
        qrep = {}       # tile -> q replicated to 128 partitions
        e2 = {}         # tile -> exp(logits), pair-expanded
        r2rep = {}      # tile -> 1/den replicated to 128 partitions
        atg_cur = None

        def emit_gather(t):
            kvsel[t] = gat.tile([128, NCC, ROW], dt.bfloat16, tag="kvsel", name="kvsel")
            g = nc.gpsimd.dma_gather(
                out_ap=kvsel[t][:], in_ap=kv_full[:, :],
                idxs_ap=idx_sb[:, ds(t * 32, 32)],
                num_idxs=QT * K, num_idxs_reg=QT * K,
                elem_size=ROW, single_packet=False)
            for cci in cc_insts:
                add_dep_helper(g.ins, cci.ins, sync=True, reason="gather after cc")

        def emit_qrep(t):
            st, g16 = t // 8, t % 8
            qrep[t] = small.tile([128, CH], dt.bfloat16, tag="qrep", name="qrep")
            for n in range(2):
                psq = ps_q.tile([128, 512], dt.float32, tag="psq")
                nc.tensor.matmul(psq[:], selrep_sb[:, g16, :],
                                 q_sb[:, st, ts(n, 512)], start=True, stop=True)
                nc.scalar.copy(qrep[t][:, ts(n, 512)], psq[:])

        def emit_stage_a(t):
            # logits: t1 = q*k, halving-tree d-reduction (pure 2x TT ops)
            t1 = big.tile([128, NCC, CH], dt.bfloat16, tag="t1")
            k_ap = kvsel[t][:, :, 0:CH]
            k_ap2, q_ap2 = bass.broadcast_tensor_aps(
                k_ap, qrep[t][:].rearrange("p (o c) -> p o c", o=1))
            nc.vector.tensor_mul(t1[:], k_ap2, q_ap2)
            t1v = t1[:].rearrange("p c (h d) -> p (c h) d", d=D)
            nc.vector.tensor_add(t1v[:, :, 0:32], t1v[:, :, 0:32], t1v[:, :, 32:64])
            nc.vector.tensor_add(t1v[:, :, 0:16], t1v[:, :, 0:16], t1v[:, :, 16:32])
            nc.vector.tensor_add(t1v[:, :, 0:8], t1v[:, :, 0:8], t1v[:, :, 8:16])
            nc.vector.tensor_add(t1v[:, :, 0:4], t1v[:, :, 0:4], t1v[:, :, 4:8])
            nc.vector.tensor_add(t1v[:, :, 0:2], t1v[:, :, 0:2], t1v[:, :, 2:4])
            lgt = small.tile([128, 4 * NH], dt.float32, tag="lgt")
            lgtv = lgt[:].rearrange("p (g o) -> p g o", o=1)
            nc.vector.tensor_add(lgtv, t1v[:, :, 0:1], t1v[:, :, 1:2])
            nc.vector.tensor_add(lgt[:], lgt[:], gb_sb[:, t, :])

            # e2 = exp(logits), pair-expanded (ACT)
            e2[t] = small.tile([128, NCC, NH, 2], dt.bfloat16, tag="e2", name="e2")
            lgt4 = lgt[:].rearrange("p (c h o) -> p c h o", c=NCC, o=1)
            e2a, lgt4b = bass.broadcast_tensor_aps(e2[t][:], lgt4)
            nc.scalar.activation(e2a, lgt4b, EXP)

            # denominator on PE (both pair lanes kept)
            psd = ps_sm.tile([16, 2 * NH], dt.float32, tag="pss")
            for cc in range(NCC):
                nc.tensor.matmul(psd[:], s16_sb[:],
                                 e2[t][:, cc, :, :].rearrange("p h w -> p (h w)"),
                                 start=(cc == 0), stop=(cc == NCC - 1))
            return psd

        def emit_recip_rep(t, psd):
            # r16 = 1/den on 16 partitions, then replicate to 128 via PE
            r16 = small.tile([16, 2 * NH], dt.float32, tag="r16")
            nc.vector.reciprocal_approx_fast(r16[:], psd[:])
            psr = ps_sm.tile([128, 2 * NH], dt.float32, tag="pss")
            nc.tensor.matmul(psr[:], s16t_sb[:], r16[:], start=True, stop=True)
            r2rep[t] = small.tile([128, 1, NH, 2], dt.bfloat16, tag="r2rep", name="r2rep")
            nc.scalar.copy(r2rep[t][:].rearrange("p o h w -> p (o h w)"), psr[:])

        def emit_stage_b_dve(t):
            # e2n = e2 * (1/den) -- normalized weights, pair-expanded
            e2n = small.tile([128, NCC, NH, 2], dt.bfloat16, tag="e2n")
            e_in, r_in = bass.broadcast_tensor_aps(e2[t][:], r2rep[t][:])
            nc.vector.tensor_mul(e2n[:], e_in, r_in)
            # W = v * e2n (bcast over d, pair-expanded 2x)
            W = big.tile([128, NCC, CH], dt.bfloat16, tag="W", name="W")
            v_ap2, e_ap2 = bass.broadcast_tensor_aps(
                kvsel[t][:, :, CH:ROW].rearrange(
                    "p c (h dd w) -> p c h dd w", dd=32, w=2),
                e2n[:].rearrange("p c h (dd w) -> p c h dd w", dd=1, w=2))
            nc.vector.tensor_mul(
                W[:].rearrange("p c (h dd w) -> p c h dd w", dd=32, w=2),
                v_ap2, e_ap2)
            return W

        def emit_stage_b_pe(t, W):
            nonlocal atg_cur
            st, g16 = t // 8, t % 8
            # AV flip: psAT[ch, q] = sum_p W[p, ch] s16[p, q]; constant
            # stationary, out free=16 -> 32 matmuls pipelining at ~30ns
            psat = ps_sm.tile([128, 8, QT], dt.float32, tag="pss")
            for chk in range(8):
                for cc in range(NCC):
                    nc.tensor.matmul(psat[:, chk, :],
                                     W[:, cc, ts(chk, 128)], s16_sb[:],
                                     start=(cc == 0), stop=(cc == NCC - 1))
            if g16 == 0:
                atg_cur = atg_pool.tile([128, 8, 128], dt.bfloat16, tag="atg",
                                        name="atg")
            nc.scalar.copy(atg_cur[:, :, ds(QT * g16, QT)], psat[:])

        oproj_q = []

        def emit_oproj_slice():
            # o-proj for a completed 8-tile group, spread over 4 iterations
            # (4 matmuls each) so the in-order PE queue never bursts ~10us
            # and delays the next tiles' psd/psr (which feed the DVE).
            if not oproj_q:
                return
            st, atg_t, psP, k = oproj_q[0]
            for chk in range(4 * k, 4 * k + 4):
                n, c8 = chk // 8, chk % 8
                nc.tensor.matmul(psP[:, ts(n, 512)], atg_t[:, c8, :],
                                 wo_sb[:, c8, ts(n, 512)],
                                 start=(c8 == 0), stop=(c8 == 7))
            if k == 3:
                oproj_q.pop(0)
                ot = outp.tile([128, H], dt.float32, tag="ot", name="ot")
                nc.scalar.copy(ot[:], psP[:])
                nc.sync.dma_start(outd[ts(st, 128), :], ot[:])
            else:
                oproj_q[0] = (st, atg_t, psP, k + 1)

        # 3-stage pipeline. Per iteration j (steady state):
        #   DVE: e2n/W(j-2) first (covers the ~0.9us gather-completion sem
        #        for tile j), then t1/tree/lgt/gb(j), then recip(j-1)
        #        (whose psd(j-1) finished an iteration ago -- no stall)
        #   PE:  qrep(j+1), psd(j), psr(j-1), psat(j-2), o-proj slice
        # so in steady state no engine head-of-line blocks on a cross-
        # engine round trip.
        psds = {}
        for t0 in range(min(1, NT)):
            emit_gather(t0)
            emit_qrep(t0)
        for i in range(NT):
            if i + 1 < NT:
                emit_gather(i + 1)
                emit_qrep(i + 1)
            W_prev = emit_stage_b_dve(i - 2) if i >= 2 else None
            psds[i] = emit_stage_a(i)
            if i >= 1:
                emit_recip_rep(i - 1, psds.pop(i - 1))
            if i >= 2:
                emit_stage_b_pe(i - 2, W_prev)
                if (i - 2) % 8 == 7:
                    psP = ps_big.tile([128, H], dt.float32, tag="psb",
                                      name="psP")
                    oproj_q.append(((i - 2) // 8, atg_cur, psP, 0))
            emit_oproj_slice()
        if NT > 0:
            emit_recip_rep(NT - 1, psds.pop(NT - 1))
            for t in (NT - 2, NT - 1):
                if t < 0:
                    continue
                W_last = emit_stage_b_dve(t)
                emit_stage_b_pe(t, W_last)
                if t % 8 == 7:
                    psP = ps_big.tile([128, H], dt.float32, tag="psb",
                                      name="psP")
                    oproj_q.append((t // 8, atg_cur, psP, 0))
            while oproj_q:
                emit_oproj_slice()

    nc.compile()
    return nc


def prep_inputs(x, idx, valid, geo_bias, Wq, Wk, Wv, Wo, bo, epoch=1):
    """Host-side shard prep. Returns (in_maps, bo_f32)."""
    x = np.asarray(x)
    idx = np.asarray(idx)
    geo_bias = np.asarray(geo_bias)
    Wq, Wk, Wv, Wo = (np.asarray(w) for w in (Wq, Wk, Wv, Wo))
    bo = np.asarray(bo, dtype=np.float32)

    x2 = x.reshape(S, H)
    scale = np.float32(1.0 / np.sqrt(D))
    w3T = np.ascontiguousarray(
        np.concatenate([(Wq * scale).T, Wk.T, Wv.T], axis=1).astype(BF16))
    woT = np.ascontiguousarray(Wo.T.astype(BF16))
    s16 = np.zeros((128, 16), dtype=BF16)
    s16[np.arange(128), np.arange(128) % 16] = 1
    s16t = np.zeros((16, 128), dtype=np.float32)
    s16t[np.arange(128) % 16, np.arange(128)] = 1
    # selrep[r, g, m] = 1 iff r == 16 g + m%16  (q replication matrices)
    selrep = np.zeros((128, 8, 128), dtype=BF16)
    for g in range(8):
        m = np.arange(128)
        selrep[16 * g + m % 16, g, m] = 1
    selrep = np.ascontiguousarray(selrep.reshape(128, 8 * 128))

    in_maps = []
    for c in range(NCORES):
        rb = c * SC
        xTc = np.ascontiguousarray(x2[rb:rb + SC].T.astype(BF16))

        # gather indices: tile t, pos = j*16 + q -> idx[rb + t*16 + q, j]
        idxc = np.empty((16, NTB * 32), dtype=np.int16)
        for t in range(NTB):
            blk = idx[rb + t * QT: rb + (t + 1) * QT, :]      # [16 q, 32 j]
            lin = np.asarray(blk.T.reshape(-1))                # pos = j*16+q
            idxc[:, t * 32:(t + 1) * 32] = lin.reshape(32, 16).T.astype(np.int16)
        idxc = np.ascontiguousarray(np.tile(idxc, (8, 1)))

        # geo bias: gb[p=(b,qq), t, cc*16+h] = geo_bias[h, rb+t*16+qq, cc*8+b]
        gg = geo_bias[:, rb:rb + SC, :]                        # [h, 512, j]
        g2 = gg.reshape(NH, NTB, QT, NCC, 8)                   # [h, t, qq, cc, b]
        gbt = g2.transpose(4, 2, 1, 3, 0).reshape(128, NTB * 4 * NH)
        gbt = np.ascontiguousarray(gbt, dtype=np.float32)

        in_maps.append({
            "xT": xTc,
            "w3T": w3T,
            "woT": woT,
            "gb": gbt,
            "idx16": idxc,
            "s16": s16,
            "s16t": s16t,
            "selrep": selrep,
        })
    return in_maps, bo


_epoch = 0


def kernel(x, idx, valid, geo_bias, Wq, Wk, Wv, Wo, bo):
    global _nc_cache, _epoch
    from concourse.bass_utils import run_bass_kernel_spmd

    if _nc_cache is None:
        _nc_cache = build_nc()
    nc = _nc_cache

    _epoch += 1
    in_maps, bo_f32 = prep_inputs(x, idx, valid, geo_bias, Wq, Wk, Wv, Wo, bo,
                                  epoch=_epoch)
    res = run_bass_kernel_spmd(nc, in_maps, core_ids=list(range(NCORES)),
                               trace=bool(int(os.environ.get("KTRACE", "0"))))
    out = np.concatenate([r["out"] for r in res.results], axis=0)
    out = out + bo_f32[None, :]
    if res.exec_time_ns is not None:
        kernel.last_exec_time_ns = res.exec_time_ns
    kernel.last_results = res
    return out.reshape(1, S, H).astype(np.float32)
